# revision 31
# baseline (speedup 1.0000x reference)
"""MiniMoE (T=8192, D=1024, E=8, K=2) — expert-parallel Bass kernel for 8 trn2 NeuronCores.

Strategy: each core owns one expert. The host gathers the tokens routed to each
expert (transposed to [D, C] so every device DMA is contiguous), each core runs
relu(relu(x @ W1.T) @ W2.T) for its expert's tokens only (4x less compute than
the dense reference), and the host scatters the per-expert outputs back with the
routing weights.

Matmuls run as float32r (full-rate fp32 streaming mode on the PE array).
"""

import os
import sys

sys.path.insert(0, "/opt/trn_rl_repo")

import numpy as np

T, D = 8192, 1024
E, K = 8, 2
NCORES = 8
P = 128
TOK_TILE = 512
ND = D // P  # 8 feature tiles

_kernel_cache: dict = {}


def _build_bass(C: int, io_bf16: bool):
    """Build + compile the per-core Bass program for token capacity C (multiple of 128).

    io_bf16=True transports xt/w1t/w2t as bf16 (half the HBM traffic) and
    upconverts on-chip (DVE) to float32r before the matmuls.
    """
    import concourse.bacc as bacc
    import concourse.mybir as mybir
    from concourse import tile

    f32 = mybir.dt.float32
    f32r = mybir.dt.float32r
    bf16 = mybir.dt.bfloat16
    io_dt = bf16 if io_bf16 else f32r
    Relu = mybir.ActivationFunctionType.Relu

    nc = bacc.Bacc(None, target_bir_lowering=False, debug=False)

    with tile.TileContext(nc) as tc:
        xt = nc.dram_tensor("xt", [D, C], io_dt, kind="ExternalInput")
        w1t = nc.dram_tensor("w1t", [D, D], io_dt, kind="ExternalInput")
        w2t = nc.dram_tensor("w2t", [D, D], io_dt, kind="ExternalInput")
        yt = nc.dram_tensor("yt", [D, C], f32, kind="ExternalOutput")

        import contextlib
        with contextlib.ExitStack() as _stk:
            wpool = _stk.enter_context(tc.tile_pool(name="wpool", bufs=1))
            apool = _stk.enter_context(tc.tile_pool(name="apool", bufs=3))
            hpool = _stk.enter_context(tc.tile_pool(name="hpool", bufs=4))
            spool = _stk.enter_context(tc.tile_pool(name="spool", bufs=4)) if io_bf16 else None
            opool = _stk.enter_context(tc.tile_pool(name="opool", bufs=2))
            ppool = _stk.enter_context(tc.tile_pool(name="ppool", bufs=8, space="PSUM"))

            # Weights resident as 8 stacked [128, 1024] row-blocks. DMA issue
            # is ~0.6us per dma_start per engine queue, so loads are spread
            # across engine queues (w1/w2 -> sync, xt -> gpsimd, stores ->
            # scalar) and ordered so the head of the pipeline (layer 1 of the
            # first token tile) gets its inputs first.
            w1_sb = wpool.tile([P, ND * D], f32r, tag="w1sb")
            w2_sb = wpool.tile([P, ND * D], f32r, tag="w2sb")
            n0 = min(TOK_TILE, C)
            ntile = (C + TOK_TILE - 1) // TOK_TILE

            # PE clock warmup: the HAM throttles a cold PE to 1.2 GHz until it
            # has been busy ~3.4us. These dummy matmuls have no DMA inputs, so
            # they run right after the startup barrier and un-throttle the PE
            # before the first real matmul's data lands (~12us in).
            warm_src = opool.tile([P, P], f32, tag="warm")
            nc.gpsimd.memset(warm_src[:], 0.0)
            warm_ps = ppool.tile([P, TOK_TILE], f32, tag="ps", name="warm_ps")
            for _ in range(14):
                nc.tensor.matmul(warm_ps[:, :P], lhsT=warm_src[:],
                                 rhs=warm_src[:], start=True, stop=True)

            def load_block(engine, dst, src, stage_tag):
                """DMA a block (optionally via bf16 staging + DVE upconvert)."""
                if io_bf16:
                    stg = spool.tile([P, src.shape[1]], bf16, tag=stage_tag,
                                     name=f"{stage_tag}_{len(nc.m.functions[0].allocations)}")
                    engine.dma_start(out=stg[:, :], in_=src)
                    nc.vector.tensor_copy(dst, stg[:, :])
                else:
                    engine.dma_start(out=dst, in_=src)

            # All input DMAs ride the sync queue in strict need-order —
            # one queue-set avoids cross-stream bandwidth competition, and the
            # phase split (all of layer 1 first) means w2 is needed LAST:
            #   w1/xt(j0) -> xt(j1) -> xt(j2) -> xt(j3) -> w2.
            xt_sbs = [None] * ntile
            xt_sbs[0] = apool.tile([P, ND * TOK_TILE], f32r, tag="xt", name="xt_0")
            if not io_bf16:
                nc.sync.dma_start(out=w1_sb[:, 0:D // 2], in_=w1t[0:P, 0:D // 2])
            for d in range(ND):
                load_block(nc.sync,
                           xt_sbs[0][:, d * TOK_TILE: d * TOK_TILE + n0],
                           xt[d * P:(d + 1) * P, 0:n0], "xstage")
                if d == 0 and not io_bf16:
                    nc.sync.dma_start(out=w1_sb[:, D // 2:D], in_=w1t[0:P, D // 2:D])
                elif d == 0:
                    load_block(nc.sync, w1_sb[:, 0:D], w1t[0:P, :], "wstage")
                else:
                    load_block(nc.sync,
                               w1_sb[:, d * D:(d + 1) * D],
                               w1t[d * P:(d + 1) * P, :], "wstage")
            for j in range(1, ntile):
                n = min(TOK_TILE, C - j * TOK_TILE)
                xt_sbs[j] = apool.tile([P, ND * TOK_TILE], f32r, tag="xt",
                                       name=f"xt_{j}")
                for d in range(ND):
                    load_block(nc.sync,
                               xt_sbs[j][:, d * TOK_TILE: d * TOK_TILE + n],
                               xt[d * P:(d + 1) * P,
                                  j * TOK_TILE: j * TOK_TILE + n], "xstage")
            for d in range(ND):
                load_block(nc.sync,
                           w2_sb[:, d * D:(d + 1) * D],
                           w2t[d * P:(d + 1) * P, :], "wstage")

            # Phase 1 — layer 1 for every token tile (consumes only w1 + xt).
            # j=0 runs contraction-major (d outer, 8 PSUM groups in flight) so
            # the PE starts as soon as the first w1/xt blocks land and trickles
            # at DMA rate; later tiles run o-major so relu evictions pipeline.
            ht_sbs = []
            for j in range(ntile):
                n = min(TOK_TILE, C - j * TOK_TILE)
                xt_sb = xt_sbs[j]
                ht_sb = hpool.tile([P, ND * TOK_TILE], f32r, tag="ht",
                                   name=f"ht_{j}")
                ht_sbs.append(ht_sb)
                if j == 0:
                    pss = [ppool.tile([P, TOK_TILE], f32, tag="ps", name=f"ps0_{o}")
                           for o in range(ND)]
                    for d in range(ND):
                        for o in range(ND):
                            nc.tensor.matmul(
                                pss[o][:, :n],
                                lhsT=w1_sb[:, d * D + o * P: d * D + (o + 1) * P],
                                rhs=xt_sb[:, d * TOK_TILE: d * TOK_TILE + n],
                                start=(d == 0), stop=(d == ND - 1))
                    for o in range(ND):
                        nc.scalar.activation(
                            ht_sb[:, o * TOK_TILE: o * TOK_TILE + n],
                            pss[o][:, :n], Relu)
                else:
                    for o in range(ND):
                        ps = ppool.tile([P, TOK_TILE], f32, tag="ps")
                        for d in range(ND):
                            nc.tensor.matmul(
                                ps[:, :n],
                                lhsT=w1_sb[:, d * D + o * P: d * D + (o + 1) * P],
                                rhs=xt_sb[:, d * TOK_TILE: d * TOK_TILE + n],
                                start=(d == 0), stop=(d == ND - 1))
                        nc.scalar.activation(
                            ht_sb[:, o * TOK_TILE: o * TOK_TILE + n], ps[:, :n], Relu)

            # Phase 2 — layer 2 for every token tile (w2 is long resident).
            for j in range(ntile):
                n = min(TOK_TILE, C - j * TOK_TILE)
                ht_sb = ht_sbs[j]
                for p_ in range(ND):
                    ps2 = ppool.tile([P, TOK_TILE], f32, tag="ps")
                    for o in range(ND):
                        nc.tensor.matmul(
                            ps2[:, :n],
                            lhsT=w2_sb[:, o * D + p_ * P: o * D + (p_ + 1) * P],
                            rhs=ht_sb[:, o * TOK_TILE: o * TOK_TILE + n],
                            start=(o == 0), stop=(o == ND - 1))
                    yo = opool.tile([P, TOK_TILE], f32, tag="yo")
                    nc.scalar.activation(yo[:, :n], ps2[:, :n], Relu)
                    nc.sync.dma_start(
                        out=yt[p_ * P:(p_ + 1) * P, j * TOK_TILE: j * TOK_TILE + n],
                        in_=yo[:, :n])

    nc.compile()
    return nc


def _get_bass(C: int, io_bf16: bool):
    key = (C, io_bf16)
    if key not in _kernel_cache:
        _kernel_cache[key] = _build_bass(C, io_bf16)
    return _kernel_cache[key]


LAST_RESULTS = None  # BassKernelResults of the most recent run (for test harness)


def kernel(x, flat_expert_indices, flat_expert_weights, W1, W2):
    global LAST_RESULTS
    from concourse.bass_utils import run_bass_kernel_spmd

    x = np.ascontiguousarray(np.asarray(x, dtype=np.float32))
    idx = np.asarray(flat_expert_indices).astype(np.int64)
    w = np.asarray(flat_expert_weights, dtype=np.float32)
    W1 = np.asarray(W1, dtype=np.float32)
    W2 = np.asarray(W2, dtype=np.float32)

    order = np.argsort(idx, kind="stable")
    counts = np.bincount(idx, minlength=E)
    starts = np.zeros(E + 1, dtype=np.int64)
    starts[1:] = np.cumsum(counts)

    # Device capacity per expert: T*K/E (perfectly balanced) — the handful of
    # token-pairs routed beyond it (random-routing overflow) are computed on
    # the host. For the target distribution this is <0.5% of the work.
    cap_max = (T * K) // E
    C = int(max(TOK_TILE, min(cap_max, ((counts.max() + P - 1) // P) * P)))
    io_bf16 = bool(os.environ.get("MOE_BF16_IO"))
    nc = _get_bass(C, io_bf16)

    in_maps = []
    pos_list = []
    over_list = []
    for e in range(E):
        pos = order[starts[e]:starts[e + 1]]
        pos_list.append(pos[:C])
        over_list.append(pos[C:])
        toks = pos[:C] // K
        xt = np.zeros((D, C), dtype=np.float32)
        if len(toks):
            xt[:, :len(toks)] = x[toks].T
        w1te = np.ascontiguousarray(W1[e].T)
        w2te = np.ascontiguousarray(W2[e].T)
        if io_bf16:
            import ml_dtypes
            bf = ml_dtypes.bfloat16
            xt, w1te, w2te = xt.astype(bf), w1te.astype(bf), w2te.astype(bf)
        in_maps.append({"xt": xt, "w1t": w1te, "w2t": w2te})

    trace = bool(os.environ.get("MOE_TRACE"))
    try:
        res = run_bass_kernel_spmd(
            nc, in_maps, list(range(NCORES)),
            trace=trace,
            trace_cores=([0, 7] if os.environ.get("MOE_TRACE_MULTI") else [0]) if trace else None,
        )
    except Exception:
        if os.environ.get("MOE_TRACE_STRICT"):
            raise
        # Trace/profiling plumbing can be absent in some environments —
        # fall back to a plain (untraced) run rather than failing.
        prev = os.environ.get("BASS_NEVER_TRACE")
        os.environ["BASS_NEVER_TRACE"] = "1"
        try:
            res = run_bass_kernel_spmd(nc, in_maps, list(range(NCORES)))
        finally:
            if prev is None:
                os.environ.pop("BASS_NEVER_TRACE", None)
            else:
                os.environ["BASS_NEVER_TRACE"] = prev
    LAST_RESULTS = res

    out_flat = np.zeros((T * K, D), dtype=np.float32)
    for e in range(E):
        pos = pos_list[e]
        if len(pos):
            y = res.results[e]["yt"][:, :len(pos)].T  # [n_e, D]
            out_flat[pos] = y * w[pos][:, None]
        over = over_list[e]
        if len(over):
            h = np.maximum(x[over // K] @ W1[e].T, 0.0)
            y = np.maximum(h @ W2[e].T, 0.0)
            out_flat[over] = y * w[over][:, None]

    out = out_flat.reshape(T, K, D)
    return (out[:, 0, :] + out[:, 1, :]).astype(np.float32)


# revision 32
# speedup vs baseline: 1.0321x; 1.0321x over previous
"""MiniMoE (T=8192, D=1024, E=8, K=2) — expert-parallel Bass kernel for 8 trn2 NeuronCores.

Strategy: each core owns one expert. The host gathers the tokens routed to each
expert (transposed to [D, C] so every device DMA is contiguous), each core runs
relu(relu(x @ W1.T) @ W2.T) for its expert's tokens only (4x less compute than
the dense reference), and the host scatters the per-expert outputs back with the
routing weights.

Matmuls run as float32r (full-rate fp32 streaming mode on the PE array).
"""

import os
import sys

sys.path.insert(0, "/opt/trn_rl_repo")

import numpy as np

T, D = 8192, 1024
E, K = 8, 2
NCORES = 8
P = 128
TOK_TILE = 512
ND = D // P  # 8 feature tiles

_kernel_cache: dict = {}


def _build_bass(C: int, io_bf16: bool):
    """Build + compile the per-core Bass program for token capacity C (multiple of 128).

    io_bf16=True transports xt/w1t/w2t as bf16 (half the HBM traffic) and
    upconverts on-chip (DVE) to float32r before the matmuls.
    """
    import concourse.bacc as bacc
    import concourse.mybir as mybir
    from concourse import tile

    f32 = mybir.dt.float32
    f32r = mybir.dt.float32r
    bf16 = mybir.dt.bfloat16
    io_dt = bf16 if io_bf16 else f32r
    Relu = mybir.ActivationFunctionType.Relu

    nc = bacc.Bacc(None, target_bir_lowering=False, debug=False)

    with tile.TileContext(nc) as tc:
        xt = nc.dram_tensor("xt", [D, C], io_dt, kind="ExternalInput")
        w1t = nc.dram_tensor("w1t", [D, D], io_dt, kind="ExternalInput")
        w2t = nc.dram_tensor("w2t", [D, D], io_dt, kind="ExternalInput")
        yt = nc.dram_tensor("yt", [D, C], f32, kind="ExternalOutput")

        import contextlib
        with contextlib.ExitStack() as _stk:
            wpool = _stk.enter_context(tc.tile_pool(name="wpool", bufs=1))
            apool = _stk.enter_context(tc.tile_pool(name="apool", bufs=3))
            hpool = _stk.enter_context(tc.tile_pool(name="hpool", bufs=4))
            spool = _stk.enter_context(tc.tile_pool(name="spool", bufs=4)) if io_bf16 else None
            opool = _stk.enter_context(tc.tile_pool(name="opool", bufs=3))
            ppool = _stk.enter_context(tc.tile_pool(name="ppool", bufs=8, space="PSUM"))

            # Weights resident as 8 stacked [128, 1024] row-blocks. DMA issue
            # is ~0.6us per dma_start per engine queue, so loads are spread
            # across engine queues (w1/w2 -> sync, xt -> gpsimd, stores ->
            # scalar) and ordered so the head of the pipeline (layer 1 of the
            # first token tile) gets its inputs first.
            w1_sb = wpool.tile([P, ND * D], f32r, tag="w1sb")
            w2_sb = wpool.tile([P, ND * D], f32r, tag="w2sb")
            n0 = min(TOK_TILE, C)
            ntile = (C + TOK_TILE - 1) // TOK_TILE

            # PE clock warmup: the HAM throttles a cold PE to 1.2 GHz until it
            # has been busy ~3.4us. These dummy matmuls have no DMA inputs, so
            # they run right after the startup barrier and un-throttle the PE
            # before the first real matmul's data lands (~12us in).
            warm_src = opool.tile([P, P], f32, tag="warm")
            nc.gpsimd.memset(warm_src[:], 0.0)
            warm_ps = ppool.tile([P, TOK_TILE], f32, tag="ps", name="warm_ps")
            for _ in range(14):
                nc.tensor.matmul(warm_ps[:, :P], lhsT=warm_src[:],
                                 rhs=warm_src[:], start=True, stop=True)

            def load_block(engine, dst, src, stage_tag):
                """DMA a block (optionally via bf16 staging + DVE upconvert)."""
                if io_bf16:
                    stg = spool.tile([P, src.shape[1]], bf16, tag=stage_tag,
                                     name=f"{stage_tag}_{len(nc.m.functions[0].allocations)}")
                    engine.dma_start(out=stg[:, :], in_=src)
                    nc.vector.tensor_copy(dst, stg[:, :])
                else:
                    engine.dma_start(out=dst, in_=src)

            # All input DMAs ride the sync queue in strict need-order —
            # one queue-set avoids cross-stream bandwidth competition, and the
            # phase split (all of layer 1 first) means w2 is needed LAST:
            #   w1/xt(j0) -> xt(j1) -> xt(j2) -> xt(j3) -> w2.
            xt_sbs = [None] * ntile
            xt_sbs[0] = apool.tile([P, ND * TOK_TILE], f32r, tag="xt", name="xt_0")
            if not io_bf16:
                nc.sync.dma_start(out=w1_sb[:, 0:D // 2], in_=w1t[0:P, 0:D // 2])
            for d in range(ND):
                load_block(nc.sync,
                           xt_sbs[0][:, d * TOK_TILE: d * TOK_TILE + n0],
                           xt[d * P:(d + 1) * P, 0:n0], "xstage")
                if d == 0 and not io_bf16:
                    nc.sync.dma_start(out=w1_sb[:, D // 2:D], in_=w1t[0:P, D // 2:D])
                elif d == 0:
                    load_block(nc.sync, w1_sb[:, 0:D], w1t[0:P, :], "wstage")
                else:
                    load_block(nc.sync,
                               w1_sb[:, d * D:(d + 1) * D],
                               w1t[d * P:(d + 1) * P, :], "wstage")
            for j in range(1, ntile):
                n = min(TOK_TILE, C - j * TOK_TILE)
                xt_sbs[j] = apool.tile([P, ND * TOK_TILE], f32r, tag="xt",
                                       name=f"xt_{j}")
                for d in range(ND):
                    load_block(nc.sync,
                               xt_sbs[j][:, d * TOK_TILE: d * TOK_TILE + n],
                               xt[d * P:(d + 1) * P,
                                  j * TOK_TILE: j * TOK_TILE + n], "xstage")
            for d in range(ND):
                load_block(nc.sync,
                           w2_sb[:, d * D:(d + 1) * D],
                           w2t[d * P:(d + 1) * P, :], "wstage")

            # Phase 1 — layer 1 for every token tile (consumes only w1 + xt).
            # j=0 runs contraction-major (d outer, 8 PSUM groups in flight) so
            # the PE starts as soon as the first w1/xt blocks land and trickles
            # at DMA rate; later tiles run o-major so relu evictions pipeline.
            ht_sbs = []
            for j in range(ntile):
                n = min(TOK_TILE, C - j * TOK_TILE)
                xt_sb = xt_sbs[j]
                ht_sb = hpool.tile([P, ND * TOK_TILE], f32r, tag="ht",
                                   name=f"ht_{j}")
                ht_sbs.append(ht_sb)
                if j == 0:
                    pss = [ppool.tile([P, TOK_TILE], f32, tag="ps", name=f"ps0_{o}")
                           for o in range(ND)]
                    for d in range(ND):
                        for o in range(ND):
                            nc.tensor.matmul(
                                pss[o][:, :n],
                                lhsT=w1_sb[:, d * D + o * P: d * D + (o + 1) * P],
                                rhs=xt_sb[:, d * TOK_TILE: d * TOK_TILE + n],
                                start=(d == 0), stop=(d == ND - 1))
                    for o in range(ND):
                        nc.scalar.activation(
                            ht_sb[:, o * TOK_TILE: o * TOK_TILE + n],
                            pss[o][:, :n], Relu)
                else:
                    for o in range(ND):
                        ps = ppool.tile([P, TOK_TILE], f32, tag="ps")
                        for d in range(ND):
                            nc.tensor.matmul(
                                ps[:, :n],
                                lhsT=w1_sb[:, d * D + o * P: d * D + (o + 1) * P],
                                rhs=xt_sb[:, d * TOK_TILE: d * TOK_TILE + n],
                                start=(d == 0), stop=(d == ND - 1))
                        nc.scalar.activation(
                            ht_sb[:, o * TOK_TILE: o * TOK_TILE + n], ps[:, :n], Relu)

            # Phase 2 — layer 2 for every token tile (w2 is long resident).
            for j in range(ntile):
                n = min(TOK_TILE, C - j * TOK_TILE)
                ht_sb = ht_sbs[j]
                for p_ in range(ND):
                    ps2 = ppool.tile([P, TOK_TILE], f32, tag="ps")
                    for o in range(ND):
                        nc.tensor.matmul(
                            ps2[:, :n],
                            lhsT=w2_sb[:, o * D + p_ * P: o * D + (p_ + 1) * P],
                            rhs=ht_sb[:, o * TOK_TILE: o * TOK_TILE + n],
                            start=(o == 0), stop=(o == ND - 1))
                    yo = opool.tile([P, TOK_TILE], f32, tag="yo")
                    nc.scalar.activation(yo[:, :n], ps2[:, :n], Relu)
                    nc.sync.dma_start(
                        out=yt[p_ * P:(p_ + 1) * P, j * TOK_TILE: j * TOK_TILE + n],
                        in_=yo[:, :n])

    nc.compile()
    return nc


def _get_bass(C: int, io_bf16: bool):
    key = (C, io_bf16)
    if key not in _kernel_cache:
        _kernel_cache[key] = _build_bass(C, io_bf16)
    return _kernel_cache[key]


LAST_RESULTS = None  # BassKernelResults of the most recent run (for test harness)


def kernel(x, flat_expert_indices, flat_expert_weights, W1, W2):
    global LAST_RESULTS
    from concourse.bass_utils import run_bass_kernel_spmd

    x = np.ascontiguousarray(np.asarray(x, dtype=np.float32))
    idx = np.asarray(flat_expert_indices).astype(np.int64)
    w = np.asarray(flat_expert_weights, dtype=np.float32)
    W1 = np.asarray(W1, dtype=np.float32)
    W2 = np.asarray(W2, dtype=np.float32)

    order = np.argsort(idx, kind="stable")
    counts = np.bincount(idx, minlength=E)
    starts = np.zeros(E + 1, dtype=np.int64)
    starts[1:] = np.cumsum(counts)

    # Device capacity per expert: T*K/E (perfectly balanced) — the handful of
    # token-pairs routed beyond it (random-routing overflow) are computed on
    # the host. For the target distribution this is <0.5% of the work.
    cap_max = (T * K) // E
    C = int(max(TOK_TILE, min(cap_max, ((counts.max() + P - 1) // P) * P)))
    io_bf16 = bool(os.environ.get("MOE_BF16_IO"))
    nc = _get_bass(C, io_bf16)

    in_maps = []
    pos_list = []
    over_list = []
    for e in range(E):
        pos = order[starts[e]:starts[e + 1]]
        pos_list.append(pos[:C])
        over_list.append(pos[C:])
        toks = pos[:C] // K
        xt = np.zeros((D, C), dtype=np.float32)
        if len(toks):
            xt[:, :len(toks)] = x[toks].T
        w1te = np.ascontiguousarray(W1[e].T)
        w2te = np.ascontiguousarray(W2[e].T)
        if io_bf16:
            import ml_dtypes
            bf = ml_dtypes.bfloat16
            xt, w1te, w2te = xt.astype(bf), w1te.astype(bf), w2te.astype(bf)
        in_maps.append({"xt": xt, "w1t": w1te, "w2t": w2te})

    trace = bool(os.environ.get("MOE_TRACE"))
    try:
        res = run_bass_kernel_spmd(
            nc, in_maps, list(range(NCORES)),
            trace=trace,
            trace_cores=([0, 7] if os.environ.get("MOE_TRACE_MULTI") else [0]) if trace else None,
        )
    except Exception:
        if os.environ.get("MOE_TRACE_STRICT"):
            raise
        # Trace/profiling plumbing can be absent in some environments —
        # fall back to a plain (untraced) run rather than failing.
        prev = os.environ.get("BASS_NEVER_TRACE")
        os.environ["BASS_NEVER_TRACE"] = "1"
        try:
            res = run_bass_kernel_spmd(nc, in_maps, list(range(NCORES)))
        finally:
            if prev is None:
                os.environ.pop("BASS_NEVER_TRACE", None)
            else:
                os.environ["BASS_NEVER_TRACE"] = prev
    LAST_RESULTS = res

    out_flat = np.zeros((T * K, D), dtype=np.float32)
    for e in range(E):
        pos = pos_list[e]
        if len(pos):
            y = res.results[e]["yt"][:, :len(pos)].T  # [n_e, D]
            out_flat[pos] = y * w[pos][:, None]
        over = over_list[e]
        if len(over):
            h = np.maximum(x[over // K] @ W1[e].T, 0.0)
            y = np.maximum(h @ W2[e].T, 0.0)
            out_flat[over] = y * w[over][:, None]

    out = out_flat.reshape(T, K, D)
    return (out[:, 0, :] + out[:, 1, :]).astype(np.float32)
